# revision 16
# baseline (speedup 1.0000x reference)
import sys, dataclasses, os
sys.path.insert(0, '/opt/trn_rl_repo')
import numpy as np
import ml_dtypes

BF = ml_dtypes.bfloat16

# dims (hardcoded per problem spec)
N, H, W, D = 2, 64, 64, 256
S = 16
NH, HD = 4, 64
MLP_H = 1024
NCORES = 8
T = 1024            # tokens per core (16 rows x 64 cols)
NT = 8              # token tiles / chunks per core
NPIX = 65 * 65      # padded image pixels
WSLOT = 8           # f32 slots: wy wx comb0-3 oy ox
EPS = 1e-6
MAGIC = 0x5f3759df

_CACHE = {}


def _bcast(ap, rep):
    return dataclasses.replace(ap, ap=[ap.ap[0], [0, rep]] + list(ap.ap[1:]))


def _dims(ap, offset_extra, dims):
    return dataclasses.replace(ap, offset=ap.offset + offset_extra,
                               ap=[ap.ap[0]] + [list(d) for d in dims])


def _build():
    import concourse.bass as bass
    import concourse.tile as tile
    from concourse import bacc, mybir

    f32 = mybir.dt.float32
    bf16 = mybir.dt.bfloat16
    i16 = mybir.dt.int16
    i32 = mybir.dt.int32
    AF = mybir.ActivationFunctionType
    OP = mybir.AluOpType
    AX = mybir.AxisListType

    nc = bacc.Bacc(None, target_bir_lowering=False, debug=False)

    def din(name, shape, dt):
        return nc.dram_tensor(name, shape, dt, kind="ExternalInput")

    hsT = din("hsT", [256, T], bf16)
    hsres = din("hsres", [T, 256], bf16)
    imgtable = din("imgtable", [NPIX, 1024], bf16)
    phA_rhs = din("phA_rhs", [256, 44], bf16)
    phA_aug = din("phA_aug", [1, 44], bf16)
    Wtil = din("Wtil", [256, 1024], bf16)
    btil = din("btil", [1, 1024], bf16)
    WvoK = din("WvoK", [1024, 256], bf16)
    wyx2 = din("wyx2", [2, 1024], bf16)
    acst = din("acst", [1, 256], bf16)
    W1p = din("W1p", [256, MLP_H], bf16)
    b1p = din("b1p", [1, MLP_H], bf16)
    W2w = din("W2w", [MLP_H, 256], bf16)
    b2a = din("b2a", [1, 256], bf16)
    s1rep = din("s1rep", [128, 256], f32)
    b1rep = din("b1rep", [128, 256], f32)
    s2rep = din("s2rep", [128, 256], f32)
    b2rep = din("b2rep", [128, 256], f32)
    ident = din("ident", [128, 128], f32)
    identb = din("identb", [128, 128], bf16)
    mask8f = din("mask8f", [128, 8], f32)
    bcmask = din("bcmask", [8, 128], f32)
    mask32 = din("mask32", [128, 32], bf16)
    refyx = din("refyx", [128, NT, 32], f32)
    btT = din("btT", [128, 8], f32)
    b1T = din("b1T", [128, 8], f32)

    out = nc.dram_tensor("out", [T, 256], f32, kind="ExternalOutput")
    wtabs = [nc.dram_tensor(f"wtab{g}", [128 * S * WSLOT], f32) for g in range(NT)]
    oftabs = [nc.dram_tensor(f"oftab{g}", [2048], i16) for g in range(NT)]

    with tile.TileContext(nc) as tc:
        with tc.tile_pool(name="const", bufs=1) as cpool:
            def ld(t, shape, dt):
                x = cpool.tile(shape, dt, tag=t.name)
                nc.sync.dma_start(x[:], t.ap())
                return x

            def ldk(t, nk, cols, dt):
                xs = []
                for k in range(nk):
                    x = cpool.tile([128, cols], dt, tag=f"{t.name}_{k}",
                                   name=f"{t.name}_{k}")
                    nc.sync.dma_start(x[:], t.ap()[128 * k:128 * (k + 1), :])
                    xs.append(x)
                return xs

            c_hsT = ldk(hsT, 2, T, bf16)
            c_rhsA = ldk(phA_rhs, 2, 44, bf16)
            c_augA = ld(phA_aug, [1, 44], bf16)
            c_Wtil = ldk(Wtil, 2, 1024, bf16)
            c_btil = ld(btil, [1, 1024], bf16)
            c_WvoK = ldk(WvoK, 8, 256, bf16)
            c_wyx2 = ld(wyx2, [2, 1024], bf16)
            c_acst = ld(acst, [1, 256], bf16)
            c_W1 = ldk(W1p, 2, MLP_H, bf16)
            c_b1 = ld(b1p, [1, MLP_H], bf16)
            c_W2 = ldk(W2w, 8, 256, bf16)
            c_b2 = ld(b2a, [1, 256], bf16)
            c_s1 = ld(s1rep, [128, 256], f32)
            c_b1r = ld(b1rep, [128, 256], f32)
            c_s2 = ld(s2rep, [128, 256], f32)
            c_b2r = ld(b2rep, [128, 256], f32)
            c_id = ld(ident, [128, 128], f32)
            c_idb = ld(identb, [128, 128], bf16)
            c_m8 = ld(mask8f, [128, 8], f32)
            c_bcm = ld(bcmask, [8, 128], f32)
            c_m32 = ld(mask32, [128, 32], bf16)
            c_ref = ld(refyx, [128, NT, 32], f32)
            c_btT = ld(btT, [128, 8], f32)
            c_b1T = ld(b1T, [128, 8], f32)

            ones_bf = cpool.tile([1, 128], bf16)
            nc.vector.memset(ones_bf[:], 1.0)

            # per-tile transposed q-tilde: qTs[g][c_lo, (a, c_hi), t_local]
            qTs = [cpool.tile([128, 8, 128], bf16, tag=f"qT{g}", name=f"qT{g}")
                   for g in range(NT)]

            with (
                tc.tile_pool(name="wrk", bufs=2) as kpool,
                tc.tile_pool(name="gat", bufs=2) as gpool,
                tc.tile_pool(name="wps", bufs=1, space="PSUM") as kpsum,
            ):
                # ---------------- stage A: offsets/weights/q ----------------
                def stageA(i):
                    psA = kpsum.tile([128, 44], f32, tag="psmA", name="psA")
                    for k in range(2):
                        nc.tensor.matmul(psA[:], c_hsT[k][:, i * 128:(i + 1) * 128],
                                         c_rhsA[k][:], start=(k == 0), stop=False)
                    nc.tensor.matmul(psA[:], ones_bf[:], c_augA[:], start=False, stop=True)

                    # th = tanh(psA/2) in [-1,1]; sigmoid = (th+1)/2
                    th = kpool.tile([128, 32], f32, tag="th")
                    nc.scalar.activation(th[:], psA[:, 0:32], AF.Tanh, scale=0.5)
                    gyxc = kpool.tile([128, 12], f32, tag="gyxc")
                    nc.vector.tensor_copy(gyxc[:], psA[:, 32:44])

                    # yyxx = 30*th + ref  (both halves at once), clip to [0,63]
                    yyxx = kpool.tile([128, 32], f32, tag="yyxx")
                    nc.vector.scalar_tensor_tensor(yyxx[:], th[:], 30.0, c_ref[:, i, :],
                                                   OP.mult, OP.add)
                    nc.vector.tensor_scalar(yyxx[:], yyxx[:], 0.0, 63.0, OP.max, OP.min)
                    yx0i = kpool.tile([128, 32], i32, tag="yx0i")
                    nc.vector.tensor_copy(yx0i[:], yyxx[:])
                    yx0 = kpool.tile([128, 32], f32, tag="yx0")
                    nc.vector.tensor_copy(yx0[:], yx0i[:])
                    fx = kpool.tile([128, 32], f32, tag="fx")
                    nc.vector.tensor_tensor(fx[:], yx0[:], yyxx[:], OP.is_gt)
                    nc.vector.tensor_tensor(yx0[:], yx0[:], fx[:], OP.subtract)
                    # w-major bridge: br[p, w, s]; rows: 0=wy 1=wx 2..5=comb 6=oy 7=ox
                    br = kpool.tile([128, WSLOT, S], f32, tag="bridge")
                    brv = br[:]
                    # rows 0,1 = wy, wx = frac part (contiguous [128,32] write)
                    nc.vector.tensor_tensor(
                        brv[:, 0:2, :].rearrange("p w s -> p (w s)"),
                        yyxx[:], yx0[:], OP.subtract)
                    # rows 6,7: oy ox = 30*th
                    nc.vector.tensor_scalar(
                        brv[:, 6:8, :].rearrange("p w s -> p (w s)"),
                        th[:], 30.0, None, OP.mult)
                    # comb_a = gy_a*oy + gx_a*ox + ct_a  (rows 2..5)
                    for a in range(4):
                        nc.vector.scalar_tensor_tensor(
                            brv[:, 2 + a, :], brv[:, 7, :], gyxc[:, 4 + a:5 + a],
                            _bcast(gyxc[:, 8 + a:9 + a], 16), OP.mult, OP.add)
                        nc.vector.scalar_tensor_tensor(
                            brv[:, 2 + a, :], brv[:, 6, :], gyxc[:, a:a + 1],
                            brv[:, 2 + a, :], OP.mult, OP.add)

                    ofsf = kpool.tile([128, 16], f32, tag="ofsf")
                    nc.vector.scalar_tensor_tensor(ofsf[:], yx0[:, 0:16], 65.0,
                                                   yx0[:, 16:32], OP.mult, OP.add)
                    oi = kpool.tile([128, 16], i16, tag="oi")
                    nc.vector.tensor_scalar(oi[:], ofsf[:], 0.0, 4158.0, OP.max, OP.min)

                    nc.sync.dma_start(
                        dataclasses.replace(wtabs[i].ap(),
                                            ap=[[S, 128], [S * 128, WSLOT], [1, S]]),
                        br[:])
                    nc.sync.dma_start(
                        dataclasses.replace(oftabs[i].ap(), ap=[[1, 128], [128, 16]]),
                        oi[:])

                    # qT[c_lo, m=(a,chi), t] = sum_d Wtil[d, a*256+chi*128+c_lo] hs[t, d]
                    psQa = kpsum.tile([128, 512], f32, tag="psH0", name="psQa")
                    psQb = kpsum.tile([128, 512], f32, tag="psH1", name="psQb")
                    for m in range(8):
                        ps = (psQa, psQb)[m // 4]
                        pd = ps[:, (m % 4) * 128:(m % 4 + 1) * 128]
                        for k in range(2):
                            nc.tensor.matmul(pd,
                                             c_Wtil[k][:, m * 128:(m + 1) * 128],
                                             c_hsT[k][:, i * 128:(i + 1) * 128],
                                             start=(k == 0), stop=(k == 1))
                    for m in range(8):
                        ps = (psQa, psQb)[m // 4]
                        nc.scalar.activation(qTs[i][:, m, :],
                                             ps[:, (m % 4) * 128:(m % 4 + 1) * 128],
                                             AF.Identity, bias=c_btT[:, m:m + 1])

                # ---------------- stage H1: WB/idx load + gathers ----------------
                def stageH1(g):
                    WB = kpool.tile([128, WSLOT, S], f32, tag="WB", bufs=3)
                    nc.sync.dma_start(
                        WB[:],
                        dataclasses.replace(
                            wtabs[g].ap(),
                            ap=[[1, 128], [S * 128, WSLOT], [128, S]]))
                    idxt = kpool.tile([128, 128], i16, tag="idxt")
                    nc.sync.dma_start(
                        idxt[:],
                        dataclasses.replace(oftabs[g].ap(),
                                            ap=[[0, 8], [128, 16], [1, 128]]))
                    gab = gpool.tile([128, 16, 1024], bf16, tag="gab")
                    for k in range(2):
                        nc.gpsimd.dma_gather(
                            gab[:, k * 8:(k + 1) * 8, :],
                            imgtable.ap(), idxt[:, k * 64:(k + 1) * 64],
                            1024, 1024, 1024)
                    return {'WB': WB, 'gab': gab}

                # ---------------- stage H2: bilinear + kvT ----------------
                def stageH2(g, st):
                    WB, gab = st['WB'], st['gab']
                    hsr = kpool.tile([128, 256], bf16, tag="hsr")
                    nc.sync.dma_start(hsr[:], hsres.ap()[g * 128:(g + 1) * 128, :])
                    st['hsr'] = hsr
                    # bilinear via 2nd-order table: row = [g0, g1=dx, g2=dy, g3=dxy]
                    # kv = (g0 + wx*g1) + wy*(g2 + wx*g3); all-bf16 ops for 2x DVE
                    wb16 = kpool.tile([128, 2, S], bf16, tag="wb16")
                    nc.vector.tensor_copy(wb16[:], WB[:, 0:2, :])
                    kv = kpool.tile([128, S, 256], bf16, tag="kv")
                    u0 = kpool.tile([128, 256], bf16, tag="u0")
                    u1 = kpool.tile([128, 256], bf16, tag="u1")
                    for j in range(S):
                        nc.vector.scalar_tensor_tensor(
                            u0[:], gab[:, j, 256:512], wb16[:, 1, j:j + 1],
                            gab[:, j, 0:256], OP.mult, OP.add)
                        nc.vector.scalar_tensor_tensor(
                            u1[:], gab[:, j, 768:1024], wb16[:, 1, j:j + 1],
                            gab[:, j, 512:768], OP.mult, OP.add)
                        nc.vector.scalar_tensor_tensor(
                            kv[:, j, :], u1[:], wb16[:, 0, j:j + 1],
                            u0[:], OP.mult, OP.add)
                    oyx2 = kpool.tile([128, 2, S], bf16, tag="oyx2")
                    nc.vector.tensor_copy(oyx2[:], WB[:, 6:8, :])
                    kvT = kpool.tile([128, 32, 128], bf16, tag="kvT")
                    psTs = [kpsum.tile([128, 512], bf16, tag=f"psT{r}", name=f"psT{r}")
                            for r in range(2)]
                    for grp in range(8):
                        psT = psTs[grp % 2]
                        for sl in range(4):
                            slab = grp * 4 + sl          # = 2*j + chi
                            j, chi = slab // 2, slab % 2
                            nc.tensor.transpose(psT[:, sl * 128:(sl + 1) * 128],
                                                kv[:, j, chi * 128:(chi + 1) * 128],
                                                c_idb[:])
                        nc.scalar.copy(kvT[:, grp * 4:(grp + 1) * 4, :], psT[:])
                    st['kv'] = kv
                    st['kvT'] = kvT
                    st['oyx2'] = oyx2

                # ---------------- helpers ----------------
                def rsqrt_nr(vr, tagp):
                    # rstd = 1/sqrt(vr) via fast-inverse-sqrt + 2 Newton steps
                    iv = kpool.tile([128, 1], i32, tag=tagp + "iv")
                    nc.vector.tensor_scalar(iv[:], vr[:].bitcast(mybir.dt.int32), 1,
                                            None, OP.logical_shift_right)
                    nc.vector.tensor_scalar(iv[:], iv[:], MAGIC, None, OP.subtract)
                    y = kpool.tile([128, 1], f32, tag=tagp + "y")
                    nc.vector.tensor_scalar(y[:].bitcast(mybir.dt.int32), iv[:], -1,
                                            None, OP.mult)
                    t = kpool.tile([128, 1], f32, tag=tagp + "t")
                    for _ in range(1):
                        nc.vector.tensor_tensor(t[:], y[:], y[:], OP.mult)
                        nc.vector.tensor_tensor(t[:], t[:], vr[:], OP.mult)
                        nc.vector.tensor_scalar(t[:], t[:], -0.5, 1.5, OP.mult, OP.add)
                        nc.vector.tensor_tensor(y[:], y[:], t[:], OP.mult)
                    return y

                def layernorm(xin, tagp):
                    sq = kpool.tile([128, 256], f32, tag="lnsq")
                    ssq = kpool.tile([128, 1], f32, tag=tagp + "ssq")
                    nc.scalar.activation(sq[:], xin[:], AF.Square, accum_out=ssq[:])
                    sm = kpool.tile([128, 1], f32, tag=tagp + "sm")
                    nc.vector.tensor_reduce(sm[:], xin[:], axis=AX.X, op=OP.add)
                    mn = kpool.tile([128, 1], f32, tag=tagp + "mn")
                    nc.vector.tensor_scalar(mn[:], sm[:], 1.0 / 256.0, None, OP.mult)
                    msq = kpool.tile([128, 1], f32, tag=tagp + "msq")
                    nc.vector.tensor_tensor(msq[:], mn[:], mn[:], OP.mult)
                    vr = kpool.tile([128, 1], f32, tag=tagp + "vr")
                    nc.vector.scalar_tensor_tensor(vr[:], ssq[:], 1.0 / 256.0, msq[:],
                                                   OP.mult, OP.subtract)
                    nc.vector.tensor_scalar(vr[:], vr[:], EPS, None, OP.add)
                    rstd = rsqrt_nr(vr, tagp)
                    xo = kpool.tile([128, 256], f32, tag=tagp + "xo")
                    nc.vector.tensor_scalar(xo[:], xin[:], mn[:], rstd[:],
                                            OP.subtract, OP.mult)
                    return xo

                # ---------------- stage T: scores..output ----------------
                def stageT(g, st):
                    WB, kv, kvT, oyx2, hsr = (st['WB'], st['kv'], st['kvT'],
                                              st['oyx2'], st['hsr'])
                    qT = qTs[g]
                    psS = kpsum.tile([128, S, 32], f32, tag="ct0", name="psS")
                    qTap = qT[:]
                    for j in range(S):
                        for chi in range(2):
                            rhs = _dims(qTap, chi * 128 + j * 8, [[1, 8], [2 * 128, 4]])
                            nc.tensor.matmul(psS[:, j, :], kvT[:, 2 * j + chi, :], rhs,
                                             start=(chi == 0), stop=(chi == 1))

                    scm = kpool.tile([128, S, 32], f32, tag="scm")
                    nc.vector.tensor_tensor(scm[:], psS[:], _bcast(c_m32[:], S), OP.mult)
                    sc = kpool.tile([128, 64], f32, tag="sc")
                    scmv = scm[:].rearrange("p j (t a) -> p j t a", a=4)
                    scmt = dataclasses.replace(
                        scmv, ap=[scmv.ap[0], scmv.ap[1], scmv.ap[3], scmv.ap[2]])
                    nc.vector.tensor_reduce(
                        sc[:].rearrange("p (j a) -> p j a", a=4), scmt, axis=AX.X, op=OP.add)
                    nc.vector.tensor_tensor(
                        sc[:].rearrange("p (j a) -> p j a", a=4),
                        sc[:].rearrange("p (j a) -> p j a", a=4),
                        _dims(WB[:], 2 * S, [[1, S], [S, 4]]), OP.add)

                    # pe_u = exp(sc) on DVE: 2^(sc*log2e), n=round(y), f in [-.5,.5]
                    # ey = sc*log2e + 127 so eni = biased exponent directly
                    ey = kpool.tile([128, 64], f32, tag="ey")
                    nc.vector.tensor_scalar(ey[:], sc[:], 1.4426950408889634,
                                            127.0, OP.mult, OP.add)
                    eni = kpool.tile([128, 64], i32, tag="eni")
                    nc.vector.tensor_copy(eni[:], ey[:])
                    enf = kpool.tile([128, 64], f32, tag="enf")
                    nc.vector.tensor_copy(enf[:], eni[:])
                    ef = kpool.tile([128, 64], f32, tag="ef")
                    nc.vector.tensor_tensor(ef[:], ey[:], enf[:], OP.subtract)
                    ep = kpool.tile([128, 64], f32, tag="ep")
                    nc.vector.tensor_scalar(ep[:], ef[:], 0.0558263,
                                            0.2401536, OP.mult, OP.add)
                    nc.vector.tensor_tensor(ep[:], ep[:], ef[:], OP.mult)
                    nc.vector.tensor_scalar(ep[:], ep[:], 0.6931471, None, OP.add)
                    nc.vector.tensor_tensor(ep[:], ep[:], ef[:], OP.mult)
                    nc.vector.tensor_scalar(ep[:], ep[:], 1.0, None, OP.add)
                    e2n = kpool.tile([128, 64], f32, tag="e2n")
                    nc.vector.tensor_scalar(e2n[:].bitcast(i32), eni[:], 23,
                                            None, OP.logical_shift_left)
                    pe_u = kpool.tile([128, 64], f32, tag="pe_u")
                    nc.vector.tensor_tensor(pe_u[:], ep[:], e2n[:], OP.mult)
                    psZ = kpsum.tile([8, 64], f32, tag="psmA", name="psZ")
                    nc.tensor.matmul(psZ[:], c_m8[:], pe_u[:], start=True, stop=True)
                    rz = kpool.tile([8, 64], f32, tag="rz")
                    nc.vector.reciprocal(rz[:], psZ[:])
                    psR = kpsum.tile([128, 64], f32, tag="psmA", name="psR")
                    nc.tensor.matmul(psR[:], c_bcm[:], rz[:], start=True, stop=True)
                    pn = kpool.tile([128, 64], bf16, tag="pn")
                    nc.vector.tensor_tensor(pn[:], pe_u[:], psR[:], OP.mult)

                    # all 16 pmj masks in one op: pmAll[p, j, (t,a)]
                    pmAll = kpool.tile([128, S, 32], bf16, tag="pmAll")
                    pnv = _dims(pn[:], 0, [[4, 16], [0, 8], [1, 4]])
                    m32v = _dims(c_m32[:], 0, [[0, 16], [4, 8], [1, 4]])
                    nc.vector.tensor_tensor(
                        pmAll[:].rearrange("p j (t a) -> p j t a", a=4), pnv, m32v, OP.mult)

                    ct0 = kpsum.tile([128, S * 32], f32, tag="ct0")
                    ct1 = kpsum.tile([128, S * 32], f32, tag="ct1")
                    ct2 = kpsum.tile([2, S * 32], f32, tag="psmA", name="ct2")
                    for j in range(S):
                        nc.tensor.matmul(ct0[:, j * 32:(j + 1) * 32], kv[:, j, 0:128],
                                         pmAll[:, j, :], start=True, stop=True)
                        nc.tensor.matmul(ct1[:, j * 32:(j + 1) * 32], kv[:, j, 128:256],
                                         pmAll[:, j, :], start=True, stop=True)
                        nc.tensor.matmul(ct2[:, j * 32:(j + 1) * 32], oyx2[:, :, j],
                                         pmAll[:, j, :], start=True, stop=True)

                    ct0s = kpool.tile([128, S * 32], bf16, tag="ct0s")
                    ct1s = kpool.tile([128, S * 32], bf16, tag="ct1s")
                    ct2s = kpool.tile([2, S * 32], bf16, tag="ct2s")
                    nc.scalar.copy(ct0s[:], ct0[:])
                    nc.scalar.copy(ct1s[:], ct1[:])
                    nc.vector.tensor_copy(ct2s[:], ct2[:])

                    psAt = kpsum.tile([128, 256], f32, tag="psmB", name="psAt")
                    first = True
                    for a in range(4):
                        for h in range(2):
                            cts = (ct0s, ct1s)[h]
                            lh = cts[:].rearrange("p (j t a) -> p (j t) a", j=S, a=4)
                            nc.tensor.matmul(psAt[:], lh[:, :, a],
                                             c_WvoK[a * 2 + h][:],
                                             start=first, stop=False)
                            first = False
                    lhp = ct2s[:].rearrange("p (j t a) -> p (j t) a", j=S, a=4)
                    for a in range(4):
                        nc.tensor.matmul(psAt[:], lhp[:, :, a],
                                         c_wyx2[:, a * 256:(a + 1) * 256],
                                         start=False, stop=False)
                    nc.tensor.matmul(psAt[:], ones_bf[:], c_acst[:], start=False, stop=True)

                    xr = kpool.tile([128, 256], f32, tag="xr")
                    nc.vector.tensor_tensor(xr[:], hsr[:], psAt[:], OP.add)

                    xh = layernorm(xr, "ln1")
                    x1 = kpool.tile([128, 256], f32, tag="x1")
                    nc.vector.tensor_tensor(x1[:], xh[:], c_s1[:], OP.mult)
                    nc.vector.tensor_tensor(x1[:], x1[:], c_b1r[:], OP.add)

                    psX = kpsum.tile([128, 256], f32, tag="psmA", name="psX")
                    nc.tensor.transpose(psX[:, 0:128], xh[:, 0:128], c_id[:])
                    nc.tensor.transpose(psX[:, 128:256], xh[:, 128:256], c_id[:])
                    xT = kpool.tile([128, 256], bf16, tag="xT")
                    nc.scalar.copy(xT[:], psX[:])

                    psH0 = kpsum.tile([128, 512], f32, tag="psH0", name="psH0")
                    psH1 = kpsum.tile([128, 512], f32, tag="psH1", name="psH1")
                    psH = [psH0, psH1]
                    for m in range(8):
                        pd = psH[m // 4][:, (m % 4) * 128:(m % 4 + 1) * 128]
                        for k in range(2):
                            nc.tensor.matmul(pd, c_W1[k][:, m * 128:(m + 1) * 128],
                                             xT[:, 128 * k:128 * (k + 1)],
                                             start=(k == 0), stop=(k == 1))
                    gh = kpool.tile([128, MLP_H], bf16, tag="gh")
                    for m in range(8):
                        hv = psH[m // 4][:, (m % 4) * 128:(m % 4 + 1) * 128]
                        nc.scalar.activation(gh[:, m * 128:(m + 1) * 128], hv,
                                             AF.Gelu_apprx_tanh,
                                             bias=c_b1T[:, m:m + 1])

                    psY = kpsum.tile([128, 256], f32, tag="psmB", name="psY")
                    for k in range(8):
                        nc.tensor.matmul(psY[:], gh[:, k * 128:(k + 1) * 128],
                                         c_W2[k][:], start=(k == 0), stop=False)
                    nc.tensor.matmul(psY[:], ones_bf[:], c_b2[:], start=False, stop=True)

                    z = kpool.tile([128, 256], f32, tag="z")
                    nc.vector.tensor_tensor(z[:], x1[:], psY[:], OP.add)
                    xh2 = layernorm(z, "ln2")
                    yout = kpool.tile([128, 256], f32, tag="yout")
                    nc.vector.tensor_tensor(yout[:], xh2[:], c_s2[:], OP.mult)
                    nc.vector.tensor_tensor(yout[:], yout[:], c_b2r[:], OP.add)
                    nc.sync.dma_start(out.ap()[g * 128:(g + 1) * 128, :], yout[:])

                # ---------------- staggered pipeline ----------------
                sts = [None] * NT
                for it in range(NT + 3):
                    if it < NT:
                        stageA(it)
                    if 1 <= it <= NT:
                        sts[it - 1] = stageH1(it - 1)
                    if 2 <= it <= NT + 1:
                        stageH2(it - 2, sts[it - 2])
                    if 3 <= it:
                        stageT(it - 3, sts[it - 3])

    nc.compile()
    return nc


def _host_prep(inputs):
    f = np.float32
    hs = np.asarray(inputs['hidden_state'], f)
    ehs = np.asarray(inputs['embedded_hidden_state'], f)
    W_off = np.asarray(inputs['W_off'], f)
    b_off = np.asarray(inputs['b_off'], f)
    W_kvp = np.asarray(inputs['W_kvp'], f)
    b_kvp = np.asarray(inputs['b_kvp'], f)
    Wq = np.asarray(inputs['Wq'], f); bq = np.asarray(inputs['bq'], f)
    Wk = np.asarray(inputs['Wk'], f); bk = np.asarray(inputs['bk'], f)
    Wv = np.asarray(inputs['Wv'], f); bv = np.asarray(inputs['bv'], f)
    Wo = np.asarray(inputs['Wo'], f); bo = np.asarray(inputs['bo'], f)
    ln1_s = np.asarray(inputs['ln1_s'], f); ln1_b = np.asarray(inputs['ln1_b'], f)
    W1 = np.asarray(inputs['W1'], f); b1 = np.asarray(inputs['b1'], f)
    W2 = np.asarray(inputs['W2'], f); b2 = np.asarray(inputs['b2'], f)
    ln2_s = np.asarray(inputs['ln2_s'], f); ln2_b = np.asarray(inputs['ln2_b'], f)

    sc = 1.0 / np.sqrt(HD)
    Wtil = np.zeros((256, 4, 256), f)
    btilv = np.zeros((4, 256), f)
    gyv = np.zeros((256, 4), f); gxv = np.zeros((256, 4), f); cv = np.zeros((256, 4), f)
    gyb = np.zeros(4, f); gxb = np.zeros(4, f); cb = np.zeros(4, f)
    WvoK = np.zeros((4, 256, 256), f)
    wyx2 = np.zeros((2, 4, 256), f)
    acst = np.array(bo, f)
    for a in range(4):
        Wt = (Wq[:, a, :] @ Wk[:, a, :].T) * sc
        bt = (bq[a] @ Wk[:, a, :].T) * sc
        Wtil[:, a, :] = Wt
        btilv[a] = bt
        gyv[:, a] = Wt @ W_kvp[0]; gyb[a] = bt @ W_kvp[0]
        gxv[:, a] = Wt @ W_kvp[1]; gxb[a] = bt @ W_kvp[1]
        kb = b_kvp @ Wk[:, a, :] + bk[a]
        cv[:, a] = (Wq[:, a, :] @ kb) * sc
        cb[a] = (bq[a] @ kb) * sc
        Wvo = Wv[:, a, :] @ Wo[a]
        WvoK[a] = Wvo
        wyx2[0, a] = W_kvp[0] @ Wvo
        wyx2[1, a] = W_kvp[1] @ Wvo
        acst = acst + (b_kvp @ Wv[:, a, :] + bv[a]) @ Wo[a]

    Woff_flat = np.concatenate(
        [W_off[:, :, 0], W_off[:, :, 1], gyv, gxv, cv], axis=1)
    baug = np.concatenate(
        [b_off[:, 0], b_off[:, 1], gyb, gxb, cb])[None, :]
    W1p = ln1_s[:, None] * W1
    b1p = (ln1_b @ W1 + b1)[None, :]

    shared = {
        'phA_rhs': Woff_flat.astype(BF), 'phA_aug': baug.astype(BF),
        'Wtil': Wtil.reshape(256, 1024).astype(BF),
        'btil': btilv.reshape(1, 1024).astype(BF),
        'WvoK': WvoK.reshape(1024, 256).astype(BF),
        'wyx2': wyx2.reshape(2, 1024).astype(BF), 'acst': acst[None, :].astype(BF),
        'W1p': W1p.astype(BF), 'b1p': b1p.astype(BF),
        'W2w': W2.astype(BF), 'b2a': b2[None, :].astype(BF),
        's1rep': np.tile(ln1_s, (128, 1)).astype(f),
        'b1rep': np.tile(ln1_b, (128, 1)).astype(f),
        's2rep': np.tile(ln2_s, (128, 1)).astype(f),
        'b2rep': np.tile(ln2_b, (128, 1)).astype(f),
        'ident': np.eye(128, dtype=f),
        'identb': np.eye(128, dtype=f).astype(BF),
        'mask8f': np.repeat(np.eye(8, dtype=f), 16, axis=0),
        'bcmask': np.repeat(np.eye(8, dtype=f), 16, axis=0).T.copy(),
        'mask32': np.repeat(np.repeat(np.eye(8, dtype=f), 16, axis=0), 4, axis=1).astype(BF),
        'btT': btilv.reshape(1024)[None, :].reshape(8, 128).T.copy().astype(f),
        'b1T': b1p.reshape(8, 128).T.copy().astype(f),
    }
    tok = np.arange(T)
    # raw grid coords (tanh form: yy = 30*th + ref)
    refy_all = (tok // 64).astype(f)          # local h in [0,16)
    refx_all = (tok % 64).astype(f)

    in_maps = []
    for c in range(NCORES):
        n, r0 = c // 4, (c % 4) * 16
        hs_c = hs[n, r0:r0 + 16].reshape(T, 256)
        img = ehs[n]
        P = np.zeros((65, 65, 256), f)
        P[:64, :64] = img
        P[64, :64] = img[63]
        P[:64, 64] = P[:64, 63]
        P[64, 64] = img[63, 63]
        yi = np.arange(65); xi = np.arange(65)
        y1 = np.minimum(yi + 1, 64); x1 = np.minimum(xi + 1, 64)
        c00 = P[yi[:, None], xi[None, :]]
        c01 = P[yi[:, None], x1[None, :]]
        c10 = P[y1[:, None], xi[None, :]]
        c11 = P[y1[:, None], x1[None, :]]
        P2 = np.concatenate([c00, c01 - c00, c10 - c00,
                             c11 - c01 - c10 + c00], axis=-1)
        m = dict(shared)
        m['hsT'] = np.ascontiguousarray(hs_c.T).astype(BF)
        m['hsres'] = hs_c.astype(BF)
        m['imgtable'] = P2.reshape(NPIX, 1024).astype(BF)
        ref = np.zeros((128, NT, 32), f)
        ry = (refy_all + r0).reshape(8, 128).T
        rx = refx_all.reshape(8, 128).T
        ref[:, :, 0:16] = ry[:, :, None]
        ref[:, :, 16:32] = rx[:, :, None]
        m['refyx'] = ref
        in_maps.append(m)
    return in_maps


def kernel(**inputs):
    from concourse.bass_utils import run_bass_kernel_spmd
    if 'nc' not in _CACHE:
        _CACHE['nc'] = _build()
    nc = _CACHE['nc']
    in_maps = _host_prep(inputs)
    res = run_bass_kernel_spmd(nc, in_maps, list(range(NCORES)))
    outs = [res.results[c]['out'].reshape(16, 64, 256) for c in range(NCORES)]
    full = np.zeros((N, H, W, D), np.float32)
    for c in range(NCORES):
        full[c // 4, (c % 4) * 16:(c % 4) * 16 + 16] = outs[c]
    return full

